# revision 3
# baseline (speedup 1.0000x reference)
"""Trainium2 Bass kernel for causal RBF (squared-exponential) attention.

  p_ij = exp(-sm * ||q_i - k_j||^2) causal-masked, out = p @ v (no normalization)
  B,H,S,D = 2,16,2048,64 ; sm = 0.125

Sharding: B*H = 32 heads, 4 heads per core across 8 NeuronCores (head
parallel, no cross-core comm).

Per-core algorithm (per head):
  - Host supplies Q^T, K^T ([D, S] layout) plus sum-of-squares rows so the
    kernel computes, per 128-key x 512-query tile, a single fp32r matmul with
    a 66-deep contraction:
        psum[kk, qq] = qk - (sq_q - 128)/2 - (sq_k - 128)/2   (= true + 128)
    rows 0..63 carry K^T/Q^T, row 64/65 carry the sum-of-square terms.
  - ScalarE computes P^T = exp(0.25 * psum - 32) straight out of PSUM into
    SBUF (groups of 3 tiles per ACTIVATE to amortize overhead).
  - Diagonal tiles are causal-masked by a VectorE multiply with a sliced
    triangular constant.
  - PV: out^T[64, 512] += V_kk^T @ P^T_kk accumulated in PSUM over kk.
  - out^T is copied to SBUF and DMA'd to DRAM in [D, S] layout; the host
    transposes back (O(N) relayout, device does all O(N^2) work).
"""

import sys

if "/opt/trn_rl_repo" not in sys.path:
    sys.path.insert(0, "/opt/trn_rl_repo")

import numpy as np

B, H, S, D = 2, 16, 2048, 64
SM = 0.125
N_CORES = 8
HPC = (B * H) // N_CORES  # heads per core = 4
SPAN = 512  # query-span per PSUM accumulation group
NSPAN = S // SPAN  # 4
KTILE = 128  # key rows per logits tile
GROUP = 3  # logits tiles per ACTIVATE (3 PSUM banks)
SQ_CENTER = 128.0  # host subtracts this from sum-of-squares rows
# psum = true_logits/(2*SM) + SQ_CENTER  ->  exp(2*SM*psum - 2*SM*SQ_CENTER)
EXP_SCALE = 2.0 * SM
EXP_BIAS = -2.0 * SM * SQ_CENTER

_CACHE = {}


def _build_module():
    """Build + compile the Bass module (once per process)."""
    if "nc" in _CACHE:
        return _CACHE["nc"]

    import concourse.mybir as mybir
    import concourse.tile as tile
    from concourse import bacc

    f32 = mybir.dt.float32
    f32r = mybir.dt.float32r

    nc = bacc.Bacc(
        "TRN2", target_bir_lowering=False, debug=False, num_devices=N_CORES
    )

    qT = nc.dram_tensor("qT", [HPC, D, S], f32r, kind="ExternalInput").ap()
    kT = nc.dram_tensor("kT", [HPC, D, S], f32r, kind="ExternalInput").ap()
    v = nc.dram_tensor("v", [HPC, S, D], f32r, kind="ExternalInput").ap()
    qsq = nc.dram_tensor("qsq", [HPC, S], f32r, kind="ExternalInput").ap()
    ksq = nc.dram_tensor("ksq", [HPC, S], f32r, kind="ExternalInput").ap()
    neghalf = nc.dram_tensor("neghalf", [1, S], f32r, kind="ExternalInput").ap()
    maskc = nc.dram_tensor("maskc", [128, 896], f32r, kind="ExternalInput").ap()
    ot = nc.dram_tensor("ot", [HPC, D, S], f32, kind="ExternalOutput").ap()

    with tile.TileContext(nc) as tc:
        with (
            tc.tile_pool(name="consts", bufs=1) as consts,
            tc.tile_pool(name="qk_sb", bufs=2) as qk_sb,
            tc.tile_pool(name="v_sb", bufs=2) as v_sb,
            tc.tile_pool(name="pt_sb", bufs=3) as pt_sb,
            tc.tile_pool(name="ot_sb", bufs=2) as ot_sb,
            tc.tile_pool(name="lg_ps", bufs=2, space="PSUM") as lg_ps,
            tc.tile_pool(name="pv_ps", bufs=2, space="PSUM") as pv_ps,
        ):
            masksb = consts.tile([128, 896], f32r, tag="mask")
            nc.sync.dma_start(out=masksb, in_=maskc)
            biassb = consts.tile([128, 1], f32, tag="bias")
            nc.vector.memset(biassb, EXP_BIAS)

            for h in range(HPC):
                qta = qk_sb.tile([D + 2, S], f32r, tag="qta")
                kta = qk_sb.tile([D + 2, S], f32r, tag="kta")
                # rows 0..63: transposed q/k; 64/65: sum-of-squares terms:
                #   kta[64] * qta[64] = ksq * (-1/2)
                #   kta[65] * qta[65] = (-1/2) * qsq
                nc.sync.dma_start(out=qta[0:D, :], in_=qT[h])
                nc.sync.dma_start(out=qta[D : D + 1, :], in_=neghalf)
                nc.sync.dma_start(out=qta[D + 1 : D + 2, :], in_=qsq[h : h + 1, :])
                nc.sync.dma_start(out=kta[0:D, :], in_=kT[h])
                nc.sync.dma_start(out=kta[D : D + 1, :], in_=ksq[h : h + 1, :])
                nc.sync.dma_start(out=kta[D + 1 : D + 2, :], in_=neghalf)

                vsb = v_sb.tile([128, S // 128, D], f32r, tag="vsb")
                nc.sync.dma_start(
                    out=vsb, in_=v[h].rearrange("(t p) d -> p t d", p=128)
                )

                for s in range(NSPAN):
                    nkk = (s + 1) * (SPAN // KTILE)  # causal: key tiles 0..nkk-1
                    po = pv_ps.tile([D, SPAN], f32, tag="po")
                    qspan = qta[:, s * SPAN : (s + 1) * SPAN]
                    for g0 in range(0, nkk, GROUP):
                        gkk = list(range(g0, min(g0 + GROUP, nkk)))
                        n = len(gkk)
                        pl = lg_ps.tile([128, GROUP, SPAN], f32, tag="pl")
                        for j, kk in enumerate(gkk):
                            nc.tensor.matmul(
                                pl[:, j, :],
                                kta[:, kk * KTILE : (kk + 1) * KTILE],
                                qspan,
                                start=True,
                                stop=True,
                            )
                        pt = pt_sb.tile([128, GROUP, SPAN], f32r, tag="pt")
                        nc.scalar.activation(
                            pt[:, 0:n, :],
                            pl[:, 0:n, :],
                            mybir.ActivationFunctionType.Exp,
                            bias=biassb,
                            scale=EXP_SCALE,
                        )
                        for j, kk in enumerate(gkk):
                            jd = kk - s * (SPAN // KTILE)
                            if jd >= 0:  # diagonal tile -> causal mask
                                c0 = 384 - 128 * jd
                                nc.vector.tensor_mul(
                                    pt[:, j, :], pt[:, j, :], masksb[:, c0 : c0 + SPAN]
                                )
                        for j, kk in enumerate(gkk):
                            nc.tensor.matmul(
                                po,
                                vsb[:, kk, :],
                                pt[:, j, :],
                                start=(kk == 0),
                                stop=(kk == nkk - 1),
                            )
                    oT = ot_sb.tile([D, SPAN], f32, tag="oT")
                    nc.vector.tensor_copy(oT, po)
                    nc.sync.dma_start(
                        out=ot[h, :, s * SPAN : (s + 1) * SPAN], in_=oT
                    )

    nc.compile()
    _CACHE["nc"] = nc
    return nc


def _host_prep(q, k, v):
    """Shard + relayout inputs for the 8 cores."""
    q = np.ascontiguousarray(np.asarray(q, dtype=np.float32)).reshape(B * H, S, D)
    k = np.ascontiguousarray(np.asarray(k, dtype=np.float32)).reshape(B * H, S, D)
    v = np.ascontiguousarray(np.asarray(v, dtype=np.float32)).reshape(B * H, S, D)

    qT = np.ascontiguousarray(q.transpose(0, 2, 1))  # [BH, D, S]
    kT = np.ascontiguousarray(k.transpose(0, 2, 1))
    qsq = (q.astype(np.float32) ** 2).sum(-1) - SQ_CENTER  # [BH, S]
    ksq = (k.astype(np.float32) ** 2).sum(-1) - SQ_CENTER

    neghalf = np.full((1, S), -0.5, dtype=np.float32)
    # maskc[r, c] = 1 if c >= r + 384 else 0 ; slice [384-128j : 896-128j]
    # gives the causal mask for diagonal tile offset j (q_local >= k_local + 128j)
    r = np.arange(128)[:, None]
    c = np.arange(896)[None, :]
    maskc = (c >= r + 384).astype(np.float32)

    in_maps = []
    for core in range(N_CORES):
        sl = slice(core * HPC, (core + 1) * HPC)
        in_maps.append(
            {
                "qT": qT[sl],
                "kT": kT[sl],
                "v": np.ascontiguousarray(v[sl]),
                "qsq": np.ascontiguousarray(qsq[sl]),
                "ksq": np.ascontiguousarray(ksq[sl]),
                "neghalf": neghalf,
                "maskc": maskc,
            }
        )
    return in_maps


def _gather(results):
    """results[core]["ot"] : [HPC, D, S] -> full [B, H, S, D]."""
    outs = [np.asarray(r["ot"]) for r in results]
    o = np.concatenate(outs, axis=0)  # [BH, D, S]
    o = o.transpose(0, 2, 1)  # [BH, S, D]
    return np.ascontiguousarray(o.reshape(B, H, S, D).astype(np.float32))


def kernel(q, k, v):
    from concourse.bass_utils import run_bass_kernel_spmd

    nc = _build_module()
    in_maps = _host_prep(q, k, v)
    res = run_bass_kernel_spmd(nc, in_maps, core_ids=list(range(N_CORES)))
    return _gather(res.results)


if __name__ == "__main__":
    rng = np.random.default_rng(0)
    q = rng.standard_normal((B, H, S, D), dtype=np.float32)
    k = rng.standard_normal((B, H, S, D), dtype=np.float32)
    v = rng.standard_normal((B, H, S, D), dtype=np.float32)
    o = kernel(q, k, v)
    print("out", o.shape, o.dtype, float(np.abs(o).max()))
